# revision 1
# baseline (speedup 1.0000x reference)
"""Trainium2 Bass kernel for MiniSelfAttention.

Shapes (full problem): x (4, 2048, 1024), Wq/Wk/Wv/Wo (1024, 1024), bo (1024,).
H=16 heads, D=64. out = softmax(q k^T / 8) v  projected by Wo.

Sharding across 8 cores: core c -> batch b = c//2, head-group g = c%2
(8 heads = 512 features per group).  Each core computes a partial output
projection (its 512 ctx features x Wo slice); host sums the two partials
per batch and adds the bias.

Per-core math (T=2048, V=1024, F=512, D=64, H=8):
  qT = (Wq_g @ x_b.T)          [F, T]   (transposed layout)
  kT = (Wk_g @ x_b.T)          [F, T]
  v  = x_b @ Wv_g.T            [T, F]   (natural layout, +ones column per head)
  per Tq block of 512, per head pair:
    scT[s, tq] = k_h.T q_h     via matmul(lhsT=kT_h[:, s-tile], rhs=qT_h[:, tq])
                               (the two heads run in separate PE row quadrants
                               via tile_position, into one [128, 1024] psum)
    eT = exp(scT / 8)          one ACT op per pair (no max subtraction:
                               scores ~ N(0,1), exp can't overflow)
    ctxT[f, tq], Z[tq] = v_aug_h.T @ eT   (ones column gives Z)
    ctxT *= 1/Z (GPSIMD partition-broadcast of Z, fast approx reciprocal)
  after each Tq block: out rows of that block = ctxT.T @ WoT_g (pipelined
  behind the next block's attention).

Matmul operands are bf16 (PSUM accumulation is fp32); the softmax
normalization and the final output stay fp32.
"""

import sys

sys.path.insert(0, "/opt/trn_rl_repo")

import numpy as np

import concourse.bacc as bacc
import concourse.mybir as mybir
from concourse import tile
from concourse.bass_utils import run_bass_kernel_spmd

F32 = mybir.dt.float32
BF16 = mybir.dt.bfloat16
AF = mybir.ActivationFunctionType

DIM = 1024
HEADS = 16
D = 64
N_CORES = 8


def build_nc(T=2048, V=DIM, F=512, mmdt=BF16):
    """Build the per-core Bass program (SPMD: same program on all cores)."""
    H = F // D                 # heads per core (8)
    KC = V // 128              # contraction chunks for projections
    NT = T // 128              # row tiles of T
    TQ = min(512, T)           # Tq block (free dim of scores matmuls)
    NJ = T // TQ               # Tq blocks
    NS = T // 128              # S (key) tiles
    KF = F // 128              # ctx feature chunks (4)
    NO = V // 512              # output column chunks (2)
    TPJ = TQ // 128            # out row-tiles per Tq block (4)
    VA = 128 * H               # v_aug: per head [v(64) | ones(1) | zeros(63)] -> M=128 + FWL

    nc = bacc.Bacc(trn_type="TRN2")
    xT = nc.dram_tensor("xT", [V, T], mmdt, kind="ExternalInput")
    wqT = nc.dram_tensor("wqT", [V, F], mmdt, kind="ExternalInput")
    wkT = nc.dram_tensor("wkT", [V, F], mmdt, kind="ExternalInput")
    wvT = nc.dram_tensor("wvT", [V, F], mmdt, kind="ExternalInput")
    woT = nc.dram_tensor("woT", [F, V], mmdt, kind="ExternalInput")
    out = nc.dram_tensor("out", [T, V], F32, kind="ExternalOutput")

    with tile.TileContext(nc) as tc:
        with (
            tc.tile_pool(name="const", bufs=1) as pconst,
            tc.tile_pool(name="persist", bufs=1) as pp,
            tc.tile_pool(name="exp", bufs=1) as pexp,
            tc.tile_pool(name="rz", bufs=1) as prz,
        ):
            ones64 = pconst.tile([1, 64], F32, tag="ones64", name="ones64")
            nc.vector.memset(ones64[:], 1.0)

            # Persistent SBUF tensors.
            qT = [pp.tile([128, T], mmdt, tag=f"qT{m}", name=f"qT{m}") for m in range(KF)]
            kT = [pp.tile([128, T], mmdt, tag=f"kT{m}", name=f"kT{m}") for m in range(KF)]
            vaug = [pp.tile([128, VA], mmdt, tag=f"va{t}", name=f"va{t}") for t in range(NT)]
            ctxT = [pp.tile([128, T], mmdt, tag=f"cT{m}", name=f"cT{m}") for m in range(KF)]
            xTs = [pp.tile([128, T], mmdt, tag=f"xT{k}", name=f"xTs{k}") for k in range(KC)]
            for k in range(KC):
                nc.sync.dma_start(xTs[k][:], xT[128 * k : 128 * (k + 1), :])

            def load_w(dram, nm):
                ws = []
                for k in range(KC):
                    w = pp.tile([128, F], mmdt, tag=f"{nm}{k}", name="w")
                    nc.sync.dma_start(w[:], dram[128 * k : 128 * (k + 1), :])
                    ws.append(w)
                return ws

            wvs = load_w(wvT, "wv")
            wqs = load_w(wqT, "wq")
            wks = load_w(wkT, "wk")
            wos = []
            for k in range(KF):
                w = pp.tile([128, V], mmdt, tag=f"wo{k}", name=f"wo{k}")
                nc.sync.dma_start(w[:], woT[128 * k : 128 * (k + 1), :])
                wos.append(w)

            # ---- phase pieces (emission order set at the bottom) ----

            def v_phase(psa):
                # v: natural layout, interleaved with ones columns
                for t in range(NT):
                    ps = psa.tile([128, F], F32, tag="mm", bufs=4, name="psmm")
                    for k in range(KC):
                        nc.tensor.matmul(
                            ps[:],
                            xTs[k][:, 128 * t : 128 * (t + 1)],
                            wvs[k][:],
                            start=(k == 0),
                            stop=(k == KC - 1),
                        )
                    nc.vector.memset(vaug[t][:], 0.0)
                    for h in range(H):
                        nc.vector.memset(vaug[t][:, 128 * h + 64 : 128 * h + 65], 1.0)
                        nc.vector.tensor_copy(
                            vaug[t][:, 128 * h : 128 * h + 64],
                            ps[:, 64 * h : 64 * (h + 1)],
                        )

            def qk_phase(m, psa):
                # qT / kT feature chunk m: out rows = q features, cols = T
                for ws, dst in ((wqs, qT), (wks, kT)):
                    for n in range(T // TQ):
                        ps = psa.tile([128, TQ], F32, tag="mm", bufs=4, name="psmm")
                        for k in range(KC):
                            nc.tensor.matmul(
                                ps[:],
                                ws[k][:, 128 * m : 128 * (m + 1)],
                                xTs[k][:, TQ * n : TQ * (n + 1)],
                                start=(k == 0),
                                stop=(k == KC - 1),
                            )
                        nc.vector.tensor_copy(dst[m][:, TQ * n : TQ * (n + 1)], ps[:])

            def attn_group(j, p):
                pcx = [
                    pps.tile([128, TQ], F32, tag="ctx", bufs=3, name="pcx")
                    for _ in range(2)
                ]
                for s in range(NS):
                    # both heads' scores side by side -> one exp op
                    sc = pps.tile([128, 2 * TQ], F32, tag="sc", bufs=2, name="sc")
                    for half in range(2):
                        lo, hi = 64 * half, 64 * half + 64
                        nc.tensor.matmul(
                            sc[:, TQ * half : TQ * (half + 1)],
                            kT[p][lo:hi, 128 * s : 128 * (s + 1)],
                            qT[p][lo:hi, TQ * j : TQ * (j + 1)],
                            tile_position=(lo, 0),
                        )
                    e = pexp.tile([128, 2 * TQ], mmdt, tag="e", bufs=3, name="e")
                    nc.scalar.activation(e[:], sc[:], AF.Exp, scale=1.0 / np.sqrt(D))
                    for half in range(2):
                        h = 2 * p + half
                        nc.tensor.matmul(
                            pcx[half][:],
                            vaug[s][:, 128 * h : 128 * (h + 1)],
                            e[:, TQ * half : TQ * (half + 1)],
                            start=(s == 0),
                            stop=(s == NS - 1),
                        )
                for half in range(2):
                    lo, hi = 64 * half, 64 * half + 64
                    # Z row -> SBUF, GPSIMD partition-broadcast to 64 rows,
                    # approx-reciprocal, then normalize the ctx rows.
                    zs = prz.tile([1, TQ], F32, tag="zs", bufs=2, name="zs")
                    nc.vector.tensor_copy(zs[:], pcx[half][64:65, :])
                    bcr = prz.tile([64, TQ], F32, tag="bcr", bufs=2, name="bcr")
                    nc.gpsimd.partition_broadcast(bcr[:], zs[:])
                    bcs = prz.tile([64, TQ], F32, tag="bcs", bufs=2, name="bcs")
                    nc.vector.reciprocal_approx_fast(bcs[:], bcr[:])
                    nc.vector.tensor_mul(
                        ctxT[p][lo:hi, TQ * j : TQ * (j + 1)],
                        pcx[half][0:64, :],
                        bcs[:],
                    )

            def out_block(j):
                # out rows for Tq block j (pipelines behind block j+1)
                for ti in range(TPJ):
                    t = TPJ * j + ti
                    ot = pp.tile([128, V], F32, tag="ot", bufs=3, name="ot")
                    for n in range(NO):
                        ps = pps.tile([128, 512], F32, tag="cmm", bufs=1, name="pscm")
                        for k in range(KF):
                            nc.tensor.matmul(
                                ps[:],
                                ctxT[k][:, 128 * t : 128 * (t + 1)],
                                wos[k][:, 512 * n : 512 * (n + 1)],
                                start=(k == 0),
                                stop=(k == KF - 1),
                            )
                        nc.vector.tensor_copy(ot[:, 512 * n : 512 * (n + 1)], ps[:])
                    nc.sync.dma_start(out[128 * t : 128 * (t + 1), :], ot[:])

            # ---- emission ----
            with tc.tile_pool(name="psA", bufs=1, space="PSUM") as psa:
                v_phase(psa)
                for m in range(KF):
                    qk_phase(m, psa)
            with tc.tile_pool(name="psB", bufs=1, space="PSUM") as pps:
                for j in range(NJ):
                    for p in range(H // 2):
                        attn_group(j, p)
                    out_block(j)

    nc.compile()
    return nc


_NC_CACHE = {}


def _get_nc(T=2048, V=DIM, F=512):
    key = (T, V, F)
    if key not in _NC_CACHE:
        _NC_CACHE[key] = build_nc(T, V, F)
    return _NC_CACHE[key]


def make_in_maps(x, Wq, Wk, Wv, Wo, np_mmdt):
    B = x.shape[0]
    F = Wq.shape[0] // 2
    in_maps = []
    for c in range(N_CORES):
        b, g = divmod(c, 2)
        rows = slice(g * F, (g + 1) * F)
        in_maps.append(
            {
                "xT": np.ascontiguousarray(x[b].T).astype(np_mmdt),
                "wqT": np.ascontiguousarray(Wq[rows].T).astype(np_mmdt),
                "wkT": np.ascontiguousarray(Wk[rows].T).astype(np_mmdt),
                "wvT": np.ascontiguousarray(Wv[rows].T).astype(np_mmdt),
                "woT": np.ascontiguousarray(Wo[:, rows].T).astype(np_mmdt),
            }
        )
    return in_maps


def kernel(x, Wq, Wk, Wv, Wo, bo, trace=False):
    x = np.asarray(x, np.float32)
    B, T, V = x.shape
    nc = _get_nc(T=T, V=V, F=V // 2)
    np_mmdt = mybir.dt.np(BF16)
    in_maps = make_in_maps(
        x,
        np.asarray(Wq, np.float32),
        np.asarray(Wk, np.float32),
        np.asarray(Wv, np.float32),
        np.asarray(Wo, np.float32),
        np_mmdt,
    )
    res = run_bass_kernel_spmd(nc, in_maps, core_ids=list(range(N_CORES)), trace=trace)
    outs = [r["out"] for r in res.results]
    full = np.empty((B, T, V), np.float32)
    for b in range(B):
        full[b] = outs[2 * b] + outs[2 * b + 1] + np.asarray(bo, np.float32)
    if trace:
        kernel.last_exec_time_ns = res.exec_time_ns
        kernel.last_results = res
    return full



# revision 11
# speedup vs baseline: 1.0852x; 1.0852x over previous
"""Trainium2 Bass kernel for MiniSelfAttention.

Shapes (full problem): x (4, 2048, 1024), Wq/Wk/Wv/Wo (1024, 1024), bo (1024,).
H=16 heads, D=64. out = softmax(q k^T / 8) v  projected by Wo.

Sharding across 8 cores: core c -> batch b = c//2, head-group g = c%2
(8 heads = 512 features per group).  Each core computes a partial output
projection (its 512 ctx features x Wo slice); host sums the two partials
per batch and adds the bias.

Schedule design (per core): the scalar engine's exp over 8 heads x T^2
scores (33.5M elems, ~1.06us per [128,1024] op, 256 ops = ~272us) is the
hard floor; the kernel keeps ACT busy back-to-back by flattening the
attention into one (pair p, q-block j, key-tile s) stream and emitting
all projection work (qkv/out matmuls) as credit-based "fillers" inside
the attention loop's PE slack.  attn@v is software-pipelined LAG tiles
behind the scores/exp stream so a late vaug tile never blocks scores.

Per-core math (T=2048, V=1024, F=512, D=64, H=8):
  kT/qT[p] = (Wk/Wq chunk p @ x.T)    [128, T]  (pair p = 2 heads)
  vaug[t]  = [v_h (64) | 1] x 8 heads [128, 520]
  scores sc[s,tq] per pair: 2 row-tiled (tile_position) matmuls, D=64 each
  e = exp(sc/8)  one ACT op per (p,j,s), no max subtraction (scores~N(0,1))
  ctx psum[65, 512] per head accumulates v_aug^T e over s; row 64 = Z
  normalize via reciprocal_approx_fast(Z) + gpsimd partition_broadcast
  out rows = ctxT.T @ WoT (emitted as fillers during last pair + tail)
"""

import sys

sys.path.insert(0, "/opt/trn_rl_repo")

from collections import deque

import numpy as np

import concourse.bacc as bacc
import concourse.mybir as mybir
from concourse import tile
from concourse.bass_utils import run_bass_kernel_spmd

F32 = mybir.dt.float32
BF16 = mybir.dt.bfloat16
AF = mybir.ActivationFunctionType

DIM = 1024
HEADS = 16
D = 64
N_CORES = 8


class Fillers:
    """Ordered queue of emission generators, drained by PE-time credit.

    A generator may carry a `gate`: it will not be pulled before the given
    global iteration.  This keeps double-buffer prefills from being emitted
    while the previous occupant still has unemitted readers (which would
    silently rebind those readers' data dependencies to the new contents).
    """

    def __init__(self):
        self.q = deque()
        self.gens = {}
        self.gates = {}

    def add(self, key, gen, gate=0):
        self.gens[key] = gen
        self.gates[key] = gate
        self.q.append(key)

    def pull(self, credit, gi=1 << 30):
        while credit > 0 and self.q:
            key = self.q[0]
            if self.gates.get(key, 0) > gi:
                return
            try:
                credit -= next(self.gens[key])
            except StopIteration:
                del self.gens[key]
                self.q.popleft()

    def force(self, key):
        gen = self.gens.pop(key, None)
        if gen is None:
            return
        for _ in gen:
            pass
        self.q.remove(key)


def build_nc(T=2048, V=DIM, F=512, mmdt=BF16):
    H = F // D                # heads per core (8)
    NP = H // 2               # head pairs (4)
    KC = V // 128             # contraction chunks (8)
    NT = T // 128             # 128-row tiles of T (16)
    TQ = 512                  # q block
    NJ = T // TQ              # q blocks (4)
    NS = T // 128             # key tiles (16)
    KF = F // 128             # ctx feature chunks (4)
    LAG = 3                   # attn@v pipeline lag (score iters)
    EB = 8                    # e-tile buffers
    CREDIT = 2.0              # filler matmul-units per attn iteration

    nc = bacc.Bacc(trn_type="TRN2")
    xT = nc.dram_tensor("xT", [V, T], mmdt, kind="ExternalInput")
    wqT = nc.dram_tensor("wqT", [V, F], mmdt, kind="ExternalInput")
    wkT = nc.dram_tensor("wkT", [V, F], mmdt, kind="ExternalInput")
    wvT = nc.dram_tensor("wvT", [V, F], mmdt, kind="ExternalInput")
    woT = nc.dram_tensor("woT", [F, V], mmdt, kind="ExternalInput")
    out = nc.dram_tensor("out", [T, V], F32, kind="ExternalOutput")

    with tile.TileContext(nc) as tc:
        with (
            tc.tile_pool(name="sb", bufs=1) as pp,
            tc.tile_pool(name="eb", bufs=1) as pe_,
            tc.tile_pool(name="rz", bufs=1) as prz,
            tc.tile_pool(name="ps", bufs=1, space="PSUM") as pps,
        ):
            # ---- persistent SBUF ----
            xTs = [pp.tile([128, T], mmdt, tag=f"xT{k}", name=f"xTs{k}") for k in range(KC)]
            wks = [pp.tile([128, F], mmdt, tag=f"wk{k}", name=f"wk{k}") for k in range(KC)]
            wqs = [pp.tile([128, F], mmdt, tag=f"wq{k}", name=f"wq{k}") for k in range(KC)]
            wvs = [pp.tile([128, F], mmdt, tag=f"wv{k}", name=f"wv{k}") for k in range(KC)]
            wos = [pp.tile([128, V], mmdt, tag=f"wo{k}", name=f"wo{k}") for k in range(KF)]
            # DMA in need-order: x (split for parallel engines) + wk feed the
            # critical kT chain, then wq, wv, wo.
            HT = T // 2
            for k in range(KC):
                nc.sync.dma_start(xTs[k][:, 0:HT], xT[128 * k : 128 * (k + 1), 0:HT])
                nc.sync.dma_start(xTs[k][:, HT:T], xT[128 * k : 128 * (k + 1), HT:T])
            for k in range(KC):
                nc.sync.dma_start(wks[k][:], wkT[128 * k : 128 * (k + 1), :])
            for k in range(KC):
                nc.sync.dma_start(wqs[k][:], wqT[128 * k : 128 * (k + 1), :])
            for k in range(KC):
                nc.sync.dma_start(wvs[k][:], wvT[128 * k : 128 * (k + 1), :])
            for k in range(KF):
                nc.sync.dma_start(wos[k][:], woT[128 * k : 128 * (k + 1), :])

            qTb = [pp.tile([128, T], mmdt, tag=f"qTb{i}", name=f"qTb{i}") for i in range(2)]
            kTb = [pp.tile([128, T], mmdt, tag=f"kTb{i}", name=f"kTb{i}") for i in range(2)]
            vaug = [pp.tile([128, 65 * H], mmdt, tag=f"va{t}", name=f"va{t}") for t in range(NT)]
            ctxT = [pp.tile([128, T], mmdt, tag=f"cT{p}", name=f"cT{p}") for p in range(NP)]

            # ---- emission generators (filler units ~ one N=512 matmul) ----
            def g_qT(p, j):
                ps = pps.tile([128, TQ], F32, tag="mm", bufs=1, name="psq")
                for k in range(KC):
                    nc.tensor.matmul(
                        ps[:],
                        wqs[k][:, 128 * p : 128 * (p + 1)],
                        xTs[k][:, TQ * j : TQ * (j + 1)],
                        start=(k == 0),
                        stop=(k == KC - 1),
                    )
                    yield 1
                nc.vector.tensor_copy(qTb[p % 2][:, TQ * j : TQ * (j + 1)], ps[:])
                yield 0.5

            def g_kT(p, n):
                ps = pps.tile([128, TQ], F32, tag="mm", bufs=1, name="psk")
                for k in range(KC):
                    nc.tensor.matmul(
                        ps[:],
                        wks[k][:, 128 * p : 128 * (p + 1)],
                        xTs[k][:, TQ * n : TQ * (n + 1)],
                        start=(k == 0),
                        stop=(k == KC - 1),
                    )
                    yield 1
                nc.vector.tensor_copy(kTb[p % 2][:, TQ * n : TQ * (n + 1)], ps[:])
                yield 0.5

            def g_v(t):
                ps = pps.tile([128, F], F32, tag="mm", bufs=1, name="psv")
                for k in range(KC):
                    nc.tensor.matmul(
                        ps[:],
                        xTs[k][:, 128 * t : 128 * (t + 1)],
                        wvs[k][:],
                        start=(k == 0),
                        stop=(k == KC - 1),
                    )
                    yield 1
                for h in range(H):
                    nc.vector.tensor_copy(
                        vaug[t][:, 65 * h : 65 * h + 64], ps[:, 64 * h : 64 * (h + 1)]
                    )
                    nc.vector.memset(vaug[t][:, 65 * h + 64 : 65 * h + 65], 1.0)
                    yield 0.35
                yield 0.2

            def g_out(t):
                ot = pp.tile([128, V], F32, tag="ot", bufs=3, name="ot")
                for n in range(2):
                    ps = pps.tile([128, 512], F32, tag="mm", bufs=1, name="pso")
                    for kf in range(KF):
                        nc.tensor.matmul(
                            ps[:],
                            ctxT[kf][:, 128 * t : 128 * (t + 1)],
                            wos[kf][:, 512 * n : 512 * (n + 1)],
                            start=(kf == 0),
                            stop=(kf == KF - 1),
                        )
                        yield 1
                    nc.vector.tensor_copy(ot[:, 512 * n : 512 * (n + 1)], ps[:])
                    yield 0.5
                # split the 512KB row-block store across DMA engines (tail tiles
                # 4-way: the very last store otherwise dominates the epilogue)
                nsp = 4 if t >= NT - 4 else 2
                w = V // nsp
                for i in range(nsp):
                    nc.sync.dma_start(
                        out[128 * t : 128 * (t + 1), w * i : w * (i + 1)],
                        ot[:, w * i : w * (i + 1)],
                    )
                yield 0.3

            fill = Fillers()

            def drain(gen):
                for _ in gen:
                    pass

            # ---- prologue: kT p0 (full), qT p0 j0 ----
            for n in range(NJ):
                drain(g_kT(0, n))
            drain(g_qT(0, 0))

            # ---- filler queue (global need-order) ----
            for t in range(8):
                fill.add(("v", t), g_v(t))
            fill.add(("qT", 0, 1), g_qT(0, 1))
            for t in range(8, 12):
                fill.add(("v", t), g_v(t))
            fill.add(("qT", 0, 2), g_qT(0, 2))
            for t in range(12, 16):
                fill.add(("v", t), g_v(t))
            fill.add(("qT", 0, 3), g_qT(0, 3))
            for p in range(1, NP):
                # pair p reuses the (p-2) buffers: do not emit its prefill
                # until group p-1 starts (all group p-2 readers emitted).
                gate = max(0, (p - 1) * NJ * NS)
                for n in range(NJ):
                    fill.add(("kT", p, n), g_kT(p, n), gate=gate)
                for j in range(NJ):
                    fill.add(("qT", p, j), g_qT(p, j), gate=gate)

            # ---- attention stream ----
            pcx_live = {}

            def emit_attnv(p, j, s, e):
                if p == 0 and j == 0:
                    fill.force(("v", s))
                if s == 0:
                    pcx_live[(p, j)] = [
                        pps.tile([65, TQ], F32, tag="cx", bufs=3, name="pcx")
                        for _ in range(2)
                    ]
                pcx = pcx_live[(p, j)]
                for half in range(2):
                    h = 2 * p + half
                    nc.tensor.matmul(
                        pcx[half][:],
                        vaug[s][:, 65 * h : 65 * h + 65],
                        e[:, TQ * half : TQ * (half + 1)],
                        start=(s == 0),
                        stop=(s == NS - 1),
                    )
                if s == NS - 1:
                    emit_norm(p, j)

            def emit_norm(p, j):
                pcx = pcx_live.pop((p, j))
                for half in range(2):
                    zs = prz.tile([1, TQ], F32, tag="zs", bufs=2, name="zs")
                    nc.vector.tensor_copy(zs[:], pcx[half][64:65, :])
                    zr = prz.tile([1, TQ], F32, tag="zr", bufs=2, name="zr")
                    nc.vector.reciprocal_approx_fast(zr[:], zs[:])
                    bc = prz.tile([64, TQ], F32, tag="bc", bufs=2, name="bc")
                    nc.gpsimd.partition_broadcast(bc[:], zr[:])
                    nc.vector.tensor_mul(
                        ctxT[p][64 * half : 64 * (half + 1), TQ * j : TQ * (j + 1)],
                        pcx[half][0:64, :],
                        bc[:],
                    )
                if p == NP - 1:
                    for ti in range(4):
                        fill.add(("out", 4 * j + ti), g_out(4 * j + ti))

            pend = deque()
            for gi in range(NP * NJ * NS):
                p, j, s = gi // (NJ * NS), (gi // NS) % NJ, gi % NS
                if s == 0:
                    if j == 0 and p > 0:
                        fill.force(("kT", p, 0))
                    fill.force(("qT", p, j))
                elif s % 4 == 0 and j == 0 and p > 0:
                    fill.force(("kT", p, s // 4))  # JIT: key block for s..s+3
                # attn@v for iteration gi-LAG first: its exp finished an
                # iteration ago, so it never stalls the PE FIFO ahead of the
                # scores that feed the next exp.
                if len(pend) >= LAG:
                    emit_attnv(*pend.popleft())
                # scores: two row-tiled 64-contraction matmuls, concurrent on PE
                sc = pps.tile([128, 2 * TQ], F32, tag="sc", bufs=2, name="sc")
                for half in range(2):
                    lo = 64 * half
                    nc.tensor.matmul(
                        sc[:, TQ * half : TQ * (half + 1)],
                        kTb[p % 2][lo : lo + 64, 128 * s : 128 * (s + 1)],
                        qTb[p % 2][lo : lo + 64, TQ * j : TQ * (j + 1)],
                        tile_position=(lo, 0),
                    )
                e = pe_.tile([128, 2 * TQ], mmdt, tag="e", bufs=EB, name="e")
                nc.scalar.activation(e[:], sc[:], AF.Exp, scale=1.0 / np.sqrt(D))
                pend.append((p, j, s, e))
                fill.pull(CREDIT, gi)
            while pend:
                emit_attnv(*pend.popleft())
            # tail: drain all remaining fillers (incl. out j3)
            while fill.q:
                fill.pull(1000.0)

    nc.compile()
    return nc


_NC_CACHE = {}


def _get_nc(T=2048, V=DIM, F=512):
    key = (T, V, F)
    if key not in _NC_CACHE:
        _NC_CACHE[key] = build_nc(T, V, F)
    return _NC_CACHE[key]


def make_in_maps(x, Wq, Wk, Wv, Wo, np_mmdt):
    B = x.shape[0]
    F = Wq.shape[0] // 2
    in_maps = []
    for c in range(N_CORES):
        b, g = divmod(c, 2)
        rows = slice(g * F, (g + 1) * F)
        in_maps.append(
            {
                "xT": np.ascontiguousarray(x[b].T).astype(np_mmdt),
                "wqT": np.ascontiguousarray(Wq[rows].T).astype(np_mmdt),
                "wkT": np.ascontiguousarray(Wk[rows].T).astype(np_mmdt),
                "wvT": np.ascontiguousarray(Wv[rows].T).astype(np_mmdt),
                "woT": np.ascontiguousarray(Wo[:, rows].T).astype(np_mmdt),
            }
        )
    return in_maps


def kernel(x, Wq, Wk, Wv, Wo, bo, trace=False):
    x = np.asarray(x, np.float32)
    B, T, V = x.shape
    nc = _get_nc(T=T, V=V, F=V // 2)
    np_mmdt = mybir.dt.np(BF16)
    in_maps = make_in_maps(
        x,
        np.asarray(Wq, np.float32),
        np.asarray(Wk, np.float32),
        np.asarray(Wv, np.float32),
        np.asarray(Wo, np.float32),
        np_mmdt,
    )
    res = run_bass_kernel_spmd(nc, in_maps, core_ids=list(range(N_CORES)), trace=trace)
    outs = [r["out"] for r in res.results]
    full = np.empty((B, T, V), np.float32)
    for b in range(B):
        full[b] = outs[2 * b] + outs[2 * b + 1] + np.asarray(bo, np.float32)
    if trace:
        kernel.last_exec_time_ns = res.exec_time_ns
        kernel.last_results = res
    return full
